# revision 11
# baseline (speedup 1.0000x reference)
"""Trainium2 Bass kernel for nn_LstmDecoder: teacher-forced LSTM decoder.

Strategy (8 NeuronCores, data-parallel over batch, B=128 -> 16/core):
  * host precompute (tiny): h0/c0 = tanh(z @ W_embed.T + b_embed),
    za_b = z @ W_ih[:,256:].T + b_ih + b_hh   (z part of the input is
    time-invariant), gate-column permutation so each 128-wide h-dim band
    holds its own [i|f|g|o] columns.
  * device, per 8-step chunk: bulk matmul gates_x = seq @ W_ih[:,:256].T
    (PE-transposed seq chunk as stationary), + za_b via GPSIMD add.
  * recurrence: per step the 4 h-dim bands of gates go to 4 PSUM
    partition bands (tile_position col groups 0/32/64/96) so the four
    M=16 matmuls overlap on the PE array; gates_x is injected into PSUM
    with an identity-columns matmul; elementwise runs once across all
    bands ([0:112] partitions, free-dim driven cost); h is transposed
    back to [h,K]-major with row-tiled PE transposes feeding both the
    next step's stationary operand and the output matmul.
  * output: per chunk out = hsT.T @ W_out.T + b_out -> DMA to DRAM.

sampled_output is sequence_input shifted by one plus a OneHotCategorical
sample of the final logits (jax threefry on host CPU, exact semantics).
"""

import os
import sys

sys.path.insert(0, "/opt/trn_rl_repo")

import numpy as np

import concourse.bass as bass
import concourse.tile as tile
from concourse import bacc, mybir
from concourse.bass_utils import run_bass_kernel_spmd

F32 = mybir.dt.float32

B, T, H, F, Z = 128, 1024, 512, 256, 256
G4 = 4 * H  # 2048
NCORES = 8
BS = B // NCORES  # 16 batch rows per core
TCH = 8  # time steps per chunk
KH = H // 128  # 4 k-chunks over hidden
KF = F // 128  # 2 k-chunks over input features
NB = 4  # h-dim bands (128 each)
BW = G4 // NB  # 512 gate-cols per band: [i|f|g|o] x 128

AF = mybir.ActivationFunctionType


def build_program(t_steps=T):
    """Build the SPMD bass program (identical on all cores)."""
    nch = t_steps // TCH
    nc = bacc.Bacc("TRN2", target_bir_lowering=False)

    # ---- DRAM I/O ----
    seq_d = nc.dram_tensor("seq", [BS, t_steps, F], F32, kind="ExternalInput")
    wst_d = nc.dram_tensor("wst", [128, KF, G4], F32, kind="ExternalInput")
    wht_d = nc.dram_tensor("wht", [128, KH, G4], F32, kind="ExternalInput")
    wot_d = nc.dram_tensor("wot", [128, KH, F], F32, kind="ExternalInput")
    zab_d = nc.dram_tensor("zab", [128, G4], F32, kind="ExternalInput")
    ht0_d = nc.dram_tensor("ht0", [128, KH, BS], F32, kind="ExternalInput")
    ct2_d = nc.dram_tensor("ct2", [128, 256], F32, kind="ExternalInput")
    i128_d = nc.dram_tensor("i128", [128, 160], F32, kind="ExternalInput")
    i16_d = nc.dram_tensor("i16", [128, 16], F32, kind="ExternalInput")
    ones_d = nc.dram_tensor("ones1", [1, 128], F32, kind="ExternalInput")
    bout_d = nc.dram_tensor("bout", [1, F], F32, kind="ExternalInput")
    out_d = nc.dram_tensor("out", [BS, t_steps, F], F32, kind="ExternalOutput")

    with tile.TileContext(nc) as tc:
        with (
            tc.tile_pool(name="singles", bufs=1) as singles,
            tc.tile_pool(name="seqp", bufs=2) as seqp,
            tc.tile_pool(name="seqtp", bufs=2) as seqtp,
            tc.tile_pool(name="gxp", bufs=3) as gxp,
            tc.tile_pool(name="hstp", bufs=2) as hstp,
            tc.tile_pool(name="outsp", bufs=2) as outsp,
            tc.tile_pool(name="gps", bufs=2, space="PSUM") as gps,
            tc.tile_pool(name="trps", bufs=2, space="PSUM") as trps,
            tc.tile_pool(name="blkps", bufs=2, space="PSUM") as blkps,
            tc.tile_pool(name="outps", bufs=1, space="PSUM") as outps,
        ):
            # ---- persistent SBUF state ----
            wst = singles.tile([128, KF, G4], F32)
            wht = singles.tile([128, KH, G4], F32)
            wot = singles.tile([128, KH, F], F32)
            zab = singles.tile([128, G4], F32)
            ht0 = singles.tile([128, KH, BS], F32)
            i128 = singles.tile([128, 160], F32)
            i16 = singles.tile([128, 16], F32)
            ones1 = singles.tile([1, 128], F32)
            bout = singles.tile([1, F], F32)
            t1 = singles.tile([128, 256], F32)  # [si | sf]
            t2 = singles.tile([128, 256], F32)  # [tg | c]
            t3 = singles.tile([128, 256], F32)  # products
            so_t = singles.tile([128, 128], F32)
            tc_t = singles.tile([128, 128], F32)
            h_t = singles.tile([128, 128], F32)

            nc.sync.dma_start(wst[:], wst_d[:].rearrange("p k g -> p (k g)"))
            nc.sync.dma_start(wht[:], wht_d[:].rearrange("p k g -> p (k g)"))
            nc.sync.dma_start(wot[:], wot_d[:].rearrange("p k g -> p (k g)"))
            nc.sync.dma_start(zab[:], zab_d[:])
            nc.sync.dma_start(ht0[:], ht0_d[:].rearrange("p k b -> p (k b)"))
            nc.sync.dma_start(t2[:], ct2_d[:])
            nc.sync.dma_start(i128[:], i128_d[:])
            nc.sync.dma_start(i16[:], i16_d[:])
            nc.sync.dma_start(ones1[:], ones_d[:])
            nc.sync.dma_start(bout[:], bout_d[:])

            hst_prev = None
            for c in range(nch):
                # ================= bulk: gates_x for this chunk ========
                seq_sb = seqp.tile([128, F], F32)
                nc.sync.dma_start(
                    seq_sb[:],
                    seq_d[:, TCH * c : TCH * (c + 1), :].rearrange(
                        "b t f -> t b f"
                    ),
                )
                seqt = seqtp.tile([128, KF, 128], F32)
                for k2 in range(KF):
                    pt = trps.tile([128, 128], F32)
                    nc.tensor.transpose(
                        pt[:], seq_sb[:, 128 * k2 : 128 * (k2 + 1)], i128[:, 0:128]
                    )
                    nc.scalar.copy(seqt[:, k2, :], pt[:])
                gx = gxp.tile([128, G4], F32)
                for n in range(4):
                    bp = blkps.tile([128, 512], F32)
                    for k2 in range(KF):
                        nc.tensor.matmul(
                            bp[:],
                            seqt[:, k2, :],
                            wst[:, k2, 512 * n : 512 * (n + 1)],
                            start=(k2 == 0),
                            stop=(k2 == KF - 1),
                        )
                    nc.vector.tensor_add(
                        gx[:, 512 * n : 512 * (n + 1)],
                        bp[:],
                        zab[:, 512 * n : 512 * (n + 1)],
                    )

                hst = hstp.tile([128, KH, 128], F32)
                # ================= 8 recurrence steps ==================
                for tl in range(TCH):
                    t = TCH * c + tl
                    g = gps.tile([128, 512], F32)
                    # inject gates_x rows for step t into the 4 bands
                    for n in range(NB):
                        nc.tensor.matmul(
                            g[32 * n : 32 * n + 32, :],
                            i128[:, 16 * tl : 16 * tl + 32],
                            gx[:, 512 * n : 512 * (n + 1)],
                            start=True,
                            stop=False,
                            tile_position=(0, 32 * n),
                            skip_group_check=True,
                        )
                    # recurrent matmul, W_hh streamed, h.T stationary
                    for k in range(KH):
                        if t == 0:
                            lhsT = ht0[:, k, :]
                        elif tl == 0:
                            lhsT = hst_prev[:, k, :].rearrange(
                                "p (b t) -> p t b", t=TCH
                            )[:, TCH - 1, :]
                        else:
                            lhsT = hst[:, k, :].rearrange(
                                "p (b t) -> p t b", t=TCH
                            )[:, tl - 1, :]
                        for n in range(NB):
                            nc.tensor.matmul(
                                g[32 * n : 32 * n + 16, :],
                                lhsT,
                                wht[:, k, 512 * n : 512 * (n + 1)],
                                start=False,
                                stop=(k == KH - 1),
                                tile_position=(0, 32 * n),
                                skip_group_check=True,
                            )
                    # elementwise across all bands (partitions 0:112)
                    P = 112
                    nc.scalar.activation(t1[0:P, :], g[0:P, 0:256], AF.Sigmoid)
                    nc.scalar.activation(
                        t2[0:P, 0:128], g[0:P, 256:384], AF.Tanh
                    )
                    nc.scalar.activation(
                        so_t[0:P, :], g[0:P, 384:512], AF.Sigmoid
                    )
                    # t3 = [si*tg | sf*c]
                    nc.vector.tensor_mul(t3[0:P, :], t1[0:P, :], t2[0:P, :])
                    # c' = si*tg + sf*c  (c lives in t2[:,128:256])
                    nc.vector.tensor_add(
                        t2[0:P, 128:256], t3[0:P, 0:128], t3[0:P, 128:256]
                    )
                    nc.scalar.activation(
                        tc_t[0:P, :], t2[0:P, 128:256], AF.Tanh
                    )
                    nc.vector.tensor_mul(h_t[0:P, :], so_t[0:P, :], tc_t[0:P, :])
                    # transpose h bands back to [h,K]-major for next step
                    for n in range(NB):
                        pt = trps.tile([128, 128], F32)
                        nc.tensor.transpose(
                            pt[:, 0:16],
                            h_t[32 * n : 32 * n + 16, :],
                            i16[32 * n : 32 * n + 16, :],
                            tile_position=(32 * n, 0),
                        )
                        nc.vector.tensor_copy(
                            hst[:, n, :].rearrange("p (b t) -> p t b", t=TCH)[
                                :, tl, :
                            ],
                            pt[:, 0:16],
                        )
                # ================= output matmul for chunk =============
                op = outps.tile([128, F], F32)
                for k in range(KH):
                    nc.tensor.matmul(
                        op[:],
                        hst[:, k, :],
                        wot[:, k, :],
                        start=(k == 0),
                        stop=False,
                    )
                nc.tensor.matmul(op[:], ones1[:], bout[:], start=False, stop=True)
                out_sb = outsp.tile([128, F], F32)
                nc.vector.tensor_copy(out_sb[:], op[:])
                nc.sync.dma_start(
                    out_d[:, TCH * c : TCH * (c + 1), :], out_sb[:]
                )
                hst_prev = hst

    nc.compile()
    return nc


# ------------------------------------------------------------------ host


def _gate_perm():
    """new gate-col layout: col' = 512*band + 128*gate + j  (band=d//128)."""
    perm = np.empty(G4, dtype=np.int64)
    for n in range(NB):
        for gi in range(4):
            j = np.arange(128)
            perm[512 * n + 128 * gi + j] = 512 * gi + 128 * n + j
    return perm


def host_precompute(z, W_embed, b_embed, W_ih, b_ih, b_hh):
    states = np.tanh(z @ W_embed.T + b_embed)  # [B, 2H]
    h0, c0 = states[:, :H], states[:, H:]
    za_b = z @ W_ih[:, F:].T + (b_ih + b_hh)  # [B, 4H]
    return h0, c0, za_b


def shared_inputs(W_ih, W_hh, W_out, b_out, perm):
    wst = np.ascontiguousarray(W_ih[:, :F].T)[:, perm]  # [256, 2048]
    wht = np.ascontiguousarray(W_hh.T)[:, perm]  # [512, 2048]
    wot = np.ascontiguousarray(W_out.T)  # [512, 256]
    d = {
        "wst": np.ascontiguousarray(
            wst.reshape(KF, 128, G4).transpose(1, 0, 2)
        ),
        "wht": np.ascontiguousarray(
            wht.reshape(KH, 128, G4).transpose(1, 0, 2)
        ),
        "wot": np.ascontiguousarray(
            wot.reshape(KH, 128, F).transpose(1, 0, 2)
        ),
        "ones1": np.ones((1, 128), np.float32),
        "bout": b_out[None, :].astype(np.float32),
    }
    i128 = np.zeros((128, 160), np.float32)
    i128[:, :128] = np.eye(128, dtype=np.float32)
    d["i128"] = i128
    i16b = np.zeros((128, 16), np.float32)
    for n in range(NB):
        i16b[32 * n : 32 * n + 16] = np.eye(16, dtype=np.float32)
    d["i16"] = i16b
    return d


def core_inputs(cid, seq, h0, c0, za_b, perm, t_steps=T):
    sl = slice(BS * cid, BS * (cid + 1))
    zs = za_b[sl][:, perm]  # [16, 2048]
    ct2 = np.zeros((128, 256), np.float32)
    ht0 = (
        np.ascontiguousarray(h0[sl].T)
        .reshape(KH, 128, BS)
        .transpose(1, 0, 2)
    )
    for n in range(NB):
        ct2[32 * n : 32 * n + 16, 128:256] = c0[sl][:, 128 * n : 128 * (n + 1)]
    return {
        "seq": np.ascontiguousarray(seq[sl, :t_steps]),
        "zab": np.tile(zs, (TCH, 1)).astype(np.float32),
        "ht0": np.ascontiguousarray(ht0),
        "ct2": ct2,
    }


_PROGRAM = None


def _get_program():
    global _PROGRAM
    if _PROGRAM is None:
        _PROGRAM = build_program(T)
    return _PROGRAM


def run_device(inputs, t_steps=T, nc=None, **spmd_kwargs):
    """Shard, run on 8 cores, gather. Returns ([B, t, F] array, results)."""
    f32 = lambda x: np.asarray(x, dtype=np.float32)
    z = f32(inputs["z"])
    seq = f32(inputs["sequence_input"])
    h0, c0, za_b = host_precompute(
        z,
        f32(inputs["W_embed"]),
        f32(inputs["b_embed"]),
        f32(inputs["W_ih"]),
        f32(inputs["b_ih"]),
        f32(inputs["b_hh"]),
    )
    perm = _gate_perm()
    shared = shared_inputs(
        f32(inputs["W_ih"]), f32(inputs["W_hh"]), f32(inputs["W_out"]),
        f32(inputs["b_out"]), perm,
    )
    in_maps = []
    for cid in range(NCORES):
        m = dict(shared)
        m.update(core_inputs(cid, seq, h0, c0, za_b, perm, t_steps))
        in_maps.append(m)
    if nc is None:
        nc = _get_program()
    res = run_bass_kernel_spmd(nc, in_maps, list(range(NCORES)), **spmd_kwargs)
    out = np.concatenate([r["out"] for r in res.results], axis=0)
    return out, res


def kernel(**inputs):
    seq_out, _ = run_device(inputs)

    # sampled_output: shifted teacher-forced inputs + one-hot sample of
    # the final logits (matches jax.random.categorical semantics).
    import jax
    import jax.numpy as jnp

    cpu = jax.devices("cpu")[0]
    with jax.default_device(cpu):
        last_logits = jnp.asarray(seq_out[:, -1]) / 1.0
        key = jax.random.fold_in(jax.random.key(0), 1234)
        cat = jax.random.categorical(key, last_logits)
        onehot = np.asarray(jax.nn.one_hot(cat, F, dtype=jnp.float32))

    seq_in = np.asarray(inputs["sequence_input"], dtype=np.float32)
    sampled = np.concatenate([seq_in[:, 1:], onehot[:, None]], axis=1)
    return seq_out, np.asarray(0.0, np.float32), sampled
